# revision 4
# baseline (speedup 1.0000x reference)
"""Trainium2 Bass kernel for modulated (StyleGAN2-style) 3x3 conv, groups=batch.

Full-input contract: kernel(**inputs) takes the unsharded numpy inputs and
returns the full (16, 512, 64, 64) fp32 output. Internally the batch is
sharded 2-per-core across 8 NeuronCores (data parallel); weight/mod params
are replicated.

Math (matching the jax reference):
    s      = style @ mod_w.T + mod_b                      # (B, IC)
    wmod   = SCALE * w * s[:,None,:,None,None]
    demod  = rsqrt(sum(wmod^2, (ic,kh,kw)) + 1e-8)        # (B, OC)
    out    = conv(x, wmod*demod, groups=batch)

Device decomposition per core (2 samples):
    conv(x, w*s) == conv(x*s, w)   -> fold s into the x fp32->bf16 pad/cast
    demod*SCALE  == rsqrt(ss + eps*IC*K*K) with ss = sum_ic WS[oc,ic]*s2[b,ic],
                    WS = sum_khkw w^2   -> one tiny PE matmul, applied as a
                    per-partition scalar on the PSUM->SBUF output copy.
    conv itself: 9 shifted bf16 matmuls x 4 ic-chunks accumulated in PSUM,
    N = 8 rows x 64 cols = 512 per matmul, x held SBUF-resident zero-padded
    to 66x66 per (sample, ic-chunk).
"""

import sys

for _p in ("/opt/trn_rl_repo",):
    if _p not in sys.path:
        sys.path.append(_p)

import numpy as np

import concourse.bass as bass
import concourse.tile as tile
from concourse import mybir
from concourse.bass_utils import run_bass_kernel_spmd

# ---------------------------------------------------------------------------
# Workaround for this container's walrus build: an instruction can carry only
# one semaphore wait (two for EventSemaphore), but Tile emits up to two per
# instruction (and the exit drain gets one per logical processor), which
# walrus rejects with "Too many sync wait commands". Fix at the BIR-JSON
# level: move excess waits onto NoOp carrier instructions inserted directly
# before the offender on the same engine — semantically identical (all waits
# still satisfied before the instruction executes, per-engine order kept).
# ---------------------------------------------------------------------------
import json as _json

_SPLIT_OK_ENGINES = {"PE", "DVE", "Activation", "Pool", "SP"}
_orig_to_json_bytes = bass.Bass.to_json_bytes


def _to_json_bytes_split_waits(self):
    raw = _orig_to_json_bytes(self)
    m = _json.loads(raw)
    changed = False
    for fn in m.get("functions", []):
        for bb in fn.get("blocks", []):
            insts = bb.get("instructions", [])
            new_insts = []
            for inst in insts:
                si = inst.get("sync_info")
                waits = (si or {}).get("on_wait") or []
                limit = 2 if inst.get("opcode") == "EventSemaphore" else 1
                if len(waits) > limit:
                    eng = inst.get("engine")
                    assert eng in _SPLIT_OK_ENGINES, (
                        f"instruction {inst.get('name')} on engine {eng} has "
                        f"{len(waits)} waits; carrier NoOp not known-safe there"
                    )
                    changed = True
                    keep = waits[-limit:]
                    for i, w in enumerate(waits[:-limit]):
                        new_insts.append(
                            {
                                "debug": inst.get("debug", 0),
                                "engine": eng,
                                "ins": [],
                                "name": f"{inst['name']}.w{i}",
                                "opcode": "NoOp",
                                "outs": [],
                                "sync_info": {"on_wait": [w], "on_update": []},
                            }
                        )
                    si["on_wait"] = keep
                new_insts.append(inst)
            bb["instructions"] = new_insts
    if not changed:
        return raw
    return _json.dumps(m).encode()


bass.Bass.to_json_bytes = _to_json_bytes_split_waits

# ---------------------------------------------------------------------------
# Problem constants (hardcoded per spec)
# ---------------------------------------------------------------------------
B, IC, OC, H, W, KS, SD = 16, 512, 512, 64, 64, 3, 512
NCORES = 8
BPC = B // NCORES          # samples per core
P = 128
NIC = IC // P              # 4 ic chunks
NOC = OC // P              # 4 oc chunks
KK = KS * KS               # 9
PW = W + 2                 # 66 padded width
RB = 8                     # output rows per block
NBLK = H // RB             # 8 blocks
NFREE = RB * W             # 512 matmul free dim
# rsqrt(SCALE^2*ss + 1e-8) * SCALE == rsqrt(ss + 1e-8*IC*K*K)
EPS_FOLDED = 1e-8 * IC * KS * KS

F32 = mybir.dt.float32
BF16 = mybir.dt.bfloat16


def build_nc():
    nc = bass.Bass()
    xs = nc.dram_tensor("xs", [BPC, IC, H, W], F32, kind="ExternalInput")
    stT = nc.dram_tensor("stT", [SD, BPC], F32, kind="ExternalInput")
    wT = nc.dram_tensor("wT", [IC, KK, OC], F32, kind="ExternalInput")
    mwT = nc.dram_tensor("mwT", [SD, IC], F32, kind="ExternalInput")
    mb = nc.dram_tensor("mb", [IC], F32, kind="ExternalInput")
    out = nc.dram_tensor("out", [BPC, OC, H, W], F32, kind="ExternalOutput")

    with tile.TileContext(nc) as tc:
        with (
            tc.tile_pool(name="singles", bufs=1) as singles,
            tc.tile_pool(name="wstage", bufs=2) as wstage,
            tc.tile_pool(name="sqp", bufs=3) as sqp,
            tc.tile_pool(name="xstage", bufs=2) as xstage,
            tc.tile_pool(name="outp", bufs=6) as outp,
            tc.tile_pool(name="psum", bufs=8, space="PSUM") as psum,
        ):
            # ---- constants ------------------------------------------------
            mwT_sb = singles.tile([P, SD // P, IC], F32)
            nc.sync.dma_start(mwT_sb, mwT.rearrange("(ko ki) i -> ki ko i", ki=P))
            stT_sb = singles.tile([P, SD // P, BPC], F32)
            nc.sync.dma_start(stT_sb, stT.rearrange("(ko ki) b -> ki ko b", ki=P))
            mb_sb = singles.tile([P, NIC], F32)
            nc.sync.dma_start(mb_sb, mb.rearrange("(c p) -> p c", p=P))

            # ---- style projection: s[ic, b] = mod_w @ style.T + mod_b -----
            s_sb = singles.tile([P, NIC, BPC], F32)
            s2_sb = singles.tile([P, NIC, BPC], F32)
            for c in range(NIC):
                ps = psum.tile([P, NFREE], F32, tag="ps")
                for k in range(SD // P):
                    nc.tensor.matmul(
                        ps[:, :BPC],
                        mwT_sb[:, k, c * P : (c + 1) * P],
                        stT_sb[:, k, :],
                        start=(k == 0),
                        stop=(k == SD // P - 1),
                    )
                nc.vector.tensor_scalar_add(s_sb[:, c, :], ps[:, :BPC], mb_sb[:, c : c + 1])
                nc.vector.tensor_mul(s2_sb[:, c, :], s_sb[:, c, :], s_sb[:, c, :])

            # ---- weights: load fp32, WS = sum_k w^2, cast to bf16 ---------
            wb = singles.tile([P, NIC, KK, OC], BF16)     # [ic, c, k, oc]
            ws = singles.tile([P, NIC, OC], F32)          # [ic, c, oc]
            for c in range(NIC):
                wst = wstage.tile([P, KK, OC], F32, tag="wst")
                nc.sync.dma_start(wst, wT[c * P : (c + 1) * P])
                nc.gpsimd.tensor_copy(out=wb[:, c], in_=wst)          # cast
                nc.scalar.square(ws[:, c, :], wst[:, 0, :])
                for k in range(1, KK):
                    sq = sqp.tile([P, OC], F32, tag="sq")
                    nc.scalar.square(sq, wst[:, k, :])
                    nc.vector.tensor_add(ws[:, c, :], ws[:, c, :], sq)

            # ---- demod[oc, b] = 1/sqrt(WS.T @ s2 + eps') ------------------
            dsq = singles.tile([P, NOC, BPC], F32)
            demod_sb = singles.tile([P, NOC, BPC], F32)
            eps_sb = singles.tile([P, 1], F32)
            nc.vector.memset(eps_sb, EPS_FOLDED)
            for o in range(NOC):
                pd = psum.tile([P, NFREE], F32, tag="ps")
                for c in range(NIC):
                    nc.tensor.matmul(
                        pd[:, :BPC],
                        ws[:, c, o * P : (o + 1) * P],
                        s2_sb[:, c, :],
                        start=(c == 0),
                        stop=(c == NIC - 1),
                    )
                nc.scalar.activation(
                    out=dsq[:, o, :],
                    in_=pd[:, :BPC],
                    func=mybir.ActivationFunctionType.Sqrt,
                    bias=eps_sb[:],
                    scale=1.0,
                )
                nc.vector.reciprocal(out=demod_sb[:, o, :], in_=dsq[:, o, :])

            # ---- x: pad to 66x66, scale by s[ic,b], cast to bf16 ----------
            xpad = singles.tile([P, BPC, NIC, PW * PW], BF16)

            def xprep(b):
                for c in range(NIC):
                    v = xpad[:, b, c, :].rearrange("p (r w) -> p r w", w=PW)
                    nc.gpsimd.memset(v[:, 0, :], 0.0)
                    nc.gpsimd.memset(v[:, PW - 1, :], 0.0)
                    nc.gpsimd.memset(v[:, 1 : PW - 1, 0:1], 0.0)
                    nc.gpsimd.memset(v[:, 1 : PW - 1, PW - 1 : PW], 0.0)
                    for r in range(0, H, H // 2):
                        xst = xstage.tile([P, H // 2, W], F32, tag="xst")
                        nc.sync.dma_start(
                            xst, xs[b, c * P : (c + 1) * P, r : r + H // 2, :]
                        )
                        nc.scalar.activation(
                            out=v[:, 1 + r : 1 + r + H // 2, 1 : 1 + W],
                            in_=xst,
                            func=mybir.ActivationFunctionType.Copy,
                            scale=s_sb[:, c, b : b + 1],
                        )

            # ---- conv: 9 shifted matmuls x 4 ic chunks --------------------
            def conv(b):
                for j in range(NBLK):
                    for o in range(NOC):
                        ps = psum.tile([P, NFREE], F32, tag="ps")
                        idx = 0
                        for ky in range(KS):
                            for kx in range(KS):
                                for c in range(NIC):
                                    xv = xpad[:, b, c, :].rearrange(
                                        "p (r w) -> p r w", w=PW
                                    )[:, j * RB + ky : j * RB + ky + RB, kx : kx + W]
                                    nc.tensor.matmul(
                                        ps,
                                        wb[:, c, ky * KS + kx, o * P : (o + 1) * P],
                                        xv,
                                        start=(idx == 0),
                                        stop=(idx == KK * NIC - 1),
                                    )
                                    idx += 1
                        ot = outp.tile([P, RB, W], F32, tag="ot")
                        nc.vector.tensor_scalar_mul(
                            ot,
                            ps.rearrange("p (r w) -> p r w", w=W),
                            demod_sb[:, o, b : b + 1],
                        )
                        nc.sync.dma_start(
                            out[b, o * P : (o + 1) * P, j * RB : (j + 1) * RB, :], ot
                        )

            xprep(0)
            conv(0)
            xprep(1)
            conv(1)

    return nc


_NC = None


def _get_nc():
    global _NC
    if _NC is None:
        _NC = build_nc()
    return _NC


def kernel(x, style, weight, mod_w, mod_b):
    x = np.ascontiguousarray(x, dtype=np.float32)
    style = np.asarray(style, dtype=np.float32)
    weight = np.asarray(weight, dtype=np.float32)
    mod_w = np.asarray(mod_w, dtype=np.float32)
    mod_b = np.ascontiguousarray(mod_b, dtype=np.float32)

    # host-side layout prep (replicated params)
    wT = np.ascontiguousarray(weight[0].transpose(1, 2, 3, 0)).reshape(IC, KK, OC)
    mwT = np.ascontiguousarray(mod_w.T)

    in_maps = []
    for i in range(NCORES):
        sl = slice(i * BPC, (i + 1) * BPC)
        in_maps.append(
            {
                "xs": np.ascontiguousarray(x[sl]),
                "stT": np.ascontiguousarray(style[sl].T),
                "wT": wT,
                "mwT": mwT,
                "mb": mod_b,
            }
        )

    nc = _get_nc()
    res = run_bass_kernel_spmd(nc, in_maps, core_ids=list(range(NCORES)))
    return np.concatenate([r["out"] for r in res.results], axis=0)


# revision 11
# speedup vs baseline: 1.1619x; 1.1619x over previous
"""Trainium2 Bass kernel for modulated (StyleGAN2-style) 3x3 conv, groups=batch.

Full-input contract: kernel(**inputs) takes the unsharded numpy inputs and
returns the full (16, 512, 64, 64) fp32 output. Internally the batch is
sharded 2-per-core across 8 NeuronCores (data parallel); weight/mod params
are replicated.

Math (matching the jax reference):
    s      = style @ mod_w.T + mod_b                      # (B, IC)
    wmod   = SCALE * w * s[:,None,:,None,None]
    demod  = rsqrt(sum(wmod^2, (ic,kh,kw)) + 1e-8)        # (B, OC)
    out    = conv(x, wmod*demod, groups=batch)

Device decomposition per core (2 samples):
    conv(x, w*s) == conv(x*s, w)   -> fold s into the x fp32->bf16 pad/cast
    demod*SCALE  == rsqrt(ss + eps*IC*K*K) with ss = sum_ic WS[oc,ic]*s2[b,ic],
                    WS = sum_khkw w^2   -> one tiny PE matmul, applied as a
                    per-partition scalar on the PSUM->SBUF output copy.
    conv itself: 9 shifted bf16 matmuls x 4 ic-chunks accumulated in PSUM,
    N = 8 rows x 64 cols = 512 per matmul, x held SBUF-resident zero-padded
    to 66x66 per (sample, ic-chunk).
"""

import sys

for _p in ("/opt/trn_rl_repo",):
    if _p not in sys.path:
        sys.path.append(_p)

import numpy as np

import concourse.bass as bass
import concourse.tile as tile
from concourse import mybir
from concourse.bass_utils import run_bass_kernel_spmd

# ---------------------------------------------------------------------------
# Workaround for this container's walrus build: an instruction can carry only
# one semaphore wait (two for EventSemaphore), but Tile emits up to two per
# instruction (and the exit drain gets one per logical processor), which
# walrus rejects with "Too many sync wait commands". Fix at the BIR-JSON
# level: move excess waits onto NoOp carrier instructions inserted directly
# before the offender on the same engine — semantically identical (all waits
# still satisfied before the instruction executes, per-engine order kept).
# ---------------------------------------------------------------------------
import json as _json

_SPLIT_OK_ENGINES = {"PE", "DVE", "Activation", "Pool", "SP"}
_orig_to_json_bytes = bass.Bass.to_json_bytes


def _to_json_bytes_split_waits(self):
    raw = _orig_to_json_bytes(self)
    m = _json.loads(raw)
    changed = False
    for fn in m.get("functions", []):
        for bb in fn.get("blocks", []):
            insts = bb.get("instructions", [])
            new_insts = []
            for inst in insts:
                si = inst.get("sync_info")
                waits = (si or {}).get("on_wait") or []
                op = inst.get("opcode", "")
                limit = 2 if op == "EventSemaphore" else 1
                if len(waits) > limit:
                    eng = inst.get("engine")
                    assert eng in _SPLIT_OK_ENGINES, (
                        f"instruction {inst.get('name')} on engine {eng} has "
                        f"{len(waits)} waits; carrier NoOp not known-safe there"
                    )
                    changed = True
                    keep = waits[-limit:]
                    for i, w in enumerate(waits[:-limit]):
                        new_insts.append(
                            {
                                "debug": inst.get("debug", 0),
                                "engine": eng,
                                "ins": [],
                                "name": f"{inst['name']}.w{i}",
                                "opcode": "NoOp",
                                "outs": [],
                                "sync_info": {"on_wait": [w], "on_update": []},
                            }
                        )
                    si["on_wait"] = keep
                new_insts.append(inst)
            bb["instructions"] = new_insts
    if not changed:
        return raw
    return _json.dumps(m).encode()


bass.Bass.to_json_bytes = _to_json_bytes_split_waits

# ---------------------------------------------------------------------------
# Problem constants (hardcoded per spec)
# ---------------------------------------------------------------------------
B, IC, OC, H, W, KS, SD = 16, 512, 512, 64, 64, 3, 512
NCORES = 8
BPC = B // NCORES          # samples per core
P = 128
NIC = IC // P              # 4 ic chunks
NOC = OC // P              # 4 oc chunks
KK = KS * KS               # 9
PW = W + 2                 # 66 padded width
RB = 8                     # output rows per block
NBLK = H // RB             # 8 blocks
NFREE = RB * W             # 512 matmul free dim
# rsqrt(SCALE^2*ss + 1e-8) * SCALE == rsqrt(ss + 1e-8*IC*K*K)
EPS_FOLDED = 1e-8 * IC * KS * KS

F32 = mybir.dt.float32
BF16 = mybir.dt.bfloat16


def build_nc():
    nc = bass.Bass()
    xs = nc.dram_tensor("xs", [BPC, IC, H, W], F32, kind="ExternalInput")
    stT = nc.dram_tensor("stT", [SD, BPC], F32, kind="ExternalInput")
    wT = nc.dram_tensor("wT", [IC, KK, OC], F32, kind="ExternalInput")
    mwT = nc.dram_tensor("mwT", [SD, IC], F32, kind="ExternalInput")
    mb = nc.dram_tensor("mb", [IC], F32, kind="ExternalInput")
    out = nc.dram_tensor("out", [BPC, OC, H, W], F32, kind="ExternalOutput")

    with tile.TileContext(nc) as tc:
        with (
            tc.tile_pool(name="singles", bufs=1) as singles,
            tc.tile_pool(name="wstage", bufs=2) as wstage,
            tc.tile_pool(name="sqp", bufs=3) as sqp,
            tc.tile_pool(name="xstage", bufs=2) as xstage,
            tc.tile_pool(name="outp", bufs=6) as outp,
            tc.tile_pool(name="psum", bufs=8, space="PSUM") as psum,
        ):
            # ---- constants ------------------------------------------------
            mwT_sb = singles.tile([P, SD // P, IC], F32)
            nc.sync.dma_start(mwT_sb, mwT.rearrange("(ko ki) i -> ki ko i", ki=P))
            stT_sb = singles.tile([P, SD // P, BPC], F32)
            nc.sync.dma_start(stT_sb, stT.rearrange("(ko ki) b -> ki ko b", ki=P))
            mb_sb = singles.tile([P, NIC], F32)
            nc.sync.dma_start(mb_sb, mb.rearrange("(c p) -> p c", p=P))

            # ---- style projection: s[ic, b] = mod_w @ style.T + mod_b -----
            s_sb = singles.tile([P, NIC, BPC], F32)
            s2_sb = singles.tile([P, NIC, BPC], F32)
            for c in range(NIC):
                ps = psum.tile([P, NFREE], F32, tag="ps")
                for k in range(SD // P):
                    nc.tensor.matmul(
                        ps[:, :BPC],
                        mwT_sb[:, k, c * P : (c + 1) * P],
                        stT_sb[:, k, :],
                        start=(k == 0),
                        stop=(k == SD // P - 1),
                    )
                nc.vector.tensor_scalar_add(s_sb[:, c, :], ps[:, :BPC], mb_sb[:, c : c + 1])
                nc.vector.tensor_mul(s2_sb[:, c, :], s_sb[:, c, :], s_sb[:, c, :])

            # ---- x: pad to 66x66, scale by s[ic,b], cast to bf16 ----------
            xpad = singles.tile([P, BPC, NIC, PW * PW], BF16)

            def xprep(b):
                for c in range(NIC):
                    v = xpad[:, b, c, :].rearrange("p (r w) -> p r w", w=PW)
                    nc.gpsimd.memset(v[:, 0, :], 0.0)
                    nc.gpsimd.memset(v[:, PW - 1, :], 0.0)
                    nc.gpsimd.memset(v[:, 1 : PW - 1, 0:1], 0.0)
                    nc.gpsimd.memset(v[:, 1 : PW - 1, PW - 1 : PW], 0.0)
                    for r in range(0, H, H // 2):
                        xst = xstage.tile([P, H // 2, W], F32, tag="xst")
                        nc.sync.dma_start(
                            xst, xs[b, c * P : (c + 1) * P, r : r + H // 2, :]
                        )
                        nc.scalar.activation(
                            out=v[:, 1 + r : 1 + r + H // 2, 1 : 1 + W],
                            in_=xst,
                            func=mybir.ActivationFunctionType.Copy,
                            scale=s_sb[:, c, b : b + 1],
                        )

            xprep(0)

            # ---- weights: load fp32, cast to bf16, WS = sum_k w^2 ---------
            # cast on DVE (GpSimd measured ~3x slower); squares on ACT;
            # the WS add chain on DVE trails behind the casts.
            wb = singles.tile([P, NIC, KK, OC], BF16)     # [ic, c, k, oc]
            ws = singles.tile([P, NIC, OC], F32)          # [ic, c, oc]
            for c in range(NIC):
                wst = wstage.tile([P, KK, OC], F32, tag="wst")
                nc.sync.dma_start(wst, wT[c * P : (c + 1) * P])
                nc.vector.tensor_copy(out=wb[:, c], in_=wst)          # cast
                nc.scalar.square(ws[:, c, :], wst[:, 0, :])
                for k in range(1, KK):
                    sq = sqp.tile([P, OC], F32, tag="sq")
                    nc.scalar.square(sq, wst[:, k, :])
                    nc.vector.tensor_add(ws[:, c, :], ws[:, c, :], sq)

            # ---- demod[oc, b] = 1/sqrt(WS.T @ s2 + eps') ------------------
            dsq = singles.tile([P, NOC, BPC], F32)
            demod_sb = singles.tile([P, NOC, BPC], F32)
            eps_sb = singles.tile([P, 1], F32)
            nc.vector.memset(eps_sb, EPS_FOLDED)
            for o in range(NOC):
                pd = psum.tile([P, NFREE], F32, tag="ps")
                for c in range(NIC):
                    nc.tensor.matmul(
                        pd[:, :BPC],
                        ws[:, c, o * P : (o + 1) * P],
                        s2_sb[:, c, :],
                        start=(c == 0),
                        stop=(c == NIC - 1),
                    )
                nc.scalar.activation(
                    out=dsq[:, o, :],
                    in_=pd[:, :BPC],
                    func=mybir.ActivationFunctionType.Sqrt,
                    bias=eps_sb[:],
                    scale=1.0,
                )
                nc.vector.reciprocal(out=demod_sb[:, o, :], in_=dsq[:, o, :])

            # ---- conv: 9 shifted matmuls x 4 ic chunks --------------------
            # ic-chunk OUTER within each accumulation group so the first
            # matmuls only need chunk 0's weights/x while later chunks are
            # still in flight from HBM.
            def conv(b):
                for j in range(NBLK):
                    for o in range(NOC):
                        ps = psum.tile([P, NFREE], F32, tag="ps")
                        idx = 0
                        for c in range(NIC):
                            for ky in range(KS):
                                for kx in range(KS):
                                    xv = xpad[:, b, c, :].rearrange(
                                        "p (r w) -> p r w", w=PW
                                    )[:, j * RB + ky : j * RB + ky + RB, kx : kx + W]
                                    nc.tensor.matmul(
                                        ps,
                                        wb[:, c, ky * KS + kx, o * P : (o + 1) * P],
                                        xv,
                                        start=(idx == 0),
                                        stop=(idx == KK * NIC - 1),
                                    )
                                    idx += 1
                        ot = outp.tile([P, RB, W], F32, tag="ot")
                        nc.vector.tensor_scalar_mul(
                            ot,
                            ps.rearrange("p (r w) -> p r w", w=W),
                            demod_sb[:, o, b : b + 1],
                        )
                        nc.sync.dma_start(
                            out[b, o * P : (o + 1) * P, j * RB : (j + 1) * RB, :], ot
                        )

            conv(0)
            xprep(1)
            conv(1)

    return nc


_NC = None


def _get_nc():
    global _NC
    if _NC is None:
        _NC = build_nc()
    return _NC


def kernel(x, style, weight, mod_w, mod_b):
    x = np.ascontiguousarray(x, dtype=np.float32)
    style = np.asarray(style, dtype=np.float32)
    weight = np.asarray(weight, dtype=np.float32)
    mod_w = np.asarray(mod_w, dtype=np.float32)
    mod_b = np.ascontiguousarray(mod_b, dtype=np.float32)

    # host-side layout prep (replicated params)
    wT = np.ascontiguousarray(weight[0].transpose(1, 2, 3, 0)).reshape(IC, KK, OC)
    mwT = np.ascontiguousarray(mod_w.T)

    in_maps = []
    for i in range(NCORES):
        sl = slice(i * BPC, (i + 1) * BPC)
        in_maps.append(
            {
                "xs": np.ascontiguousarray(x[sl]),
                "stT": np.ascontiguousarray(style[sl].T),
                "wT": wT,
                "mwT": mwT,
                "mb": mod_b,
            }
        )

    nc = _get_nc()
    res = run_bass_kernel_spmd(nc, in_maps, core_ids=list(range(NCORES)))
    return np.concatenate([r["out"] for r in res.results], axis=0)
